# revision 1
# baseline (speedup 1.0000x reference)
"""Trainium2 Bass kernel for nn_DiffusionLayer_rec2_transformer (point-transformer
layer: KNN-16 attention over 8192 points, batch 2, 128 channels).

Self-contained: kernel(**inputs) -> np.ndarray [2, 128, 8192].

Distribution: 8 NeuronCores; core c handles batch c//4, query slice
(c%4)*2048. Each core receives its batch's full point set (column-rotated so
its own queries sit at columns 0..2047) and computes KNN + attention for its
2048 queries; GroupNorm statistics are combined across the 4 cores of each
batch with tiny AllReduces.

KNN exactness: coarse scores via an fp16-pair K=13 matmul (error ~1e-5),
per-512-chunk top-8 (DVE max8) + top-24 merge, then exact-fp32 refinement of
the 24 candidates from squared coordinate differences; verified to reproduce
jax fp32 top-16 sets exactly for this problem's data distribution.
"""
import dataclasses
import numpy as np
import concourse.bass as bass
import concourse.bacc as bacc
import concourse.tile as tile
from concourse import mybir
from concourse.bass_utils import run_bass_kernel_spmd

dt = mybir.dt
AF = mybir.ActivationFunctionType
ALU = mybir.AluOpType
AX = mybir.AxisListType

N = 8192
NQ = 2048
K = 16
CHUNK = 512
NCH = N // CHUNK          # 16 chunks
NCAND = 24
BIG = 1e30
F16BIG = 60000.0
EPS = 1e-5
NEG = 0.1


def build(n_cores=8, ntiles=16, groups=None, dbg=(), group_size=4):
    if groups is None:
        groups = [[0, 1, 2, 3], [4, 5, 6, 7]] if n_cores == 8 else [[c] for c in range(n_cores)]
    nc = bacc.Bacc("TRN2", target_bir_lowering=False, debug=False,
                   num_devices=n_cores)

    def din(name, shape, d=dt.float32):
        return nc.dram_tensor(name, shape, d, kind="ExternalInput")

    # ---- inputs (per-core, host-prepped; see npmodel.host_prep) ----
    feat = din("feat", [128, N])
    rhs13 = din("rhs13", [16, N], dt.float16)
    lhsT13 = din("lhsT13", [16, NQ], dt.float16)
    cT3 = din("cT3", [4, NQ])
    pcT = din("pcT", [NQ, 4])
    xyzrow = din("xyzrow", [4, N])
    wnames = ["LWpre", "LWq", "LWkneg", "LWv", "LWpos2a", "LWpos2b",
              "LWatt1", "LWatt2a", "LWatt2b", "LWpost"]
    W = {n_: din(n_, [128, 128]) for n_ in wnames}
    lhsT6 = din("lhsT6", [6, 128])
    bnames = ["Bpre", "Bv", "Battin", "Batt1", "Batt2", "Bpost", "Bpos1",
              "Gpos", "BEpos", "Gatt", "BEatt", "Gpost", "BEpost"]
    B = {n_: din(n_, [128, 1]) for n_ in bnames}
    ident = din("ident", [128, 128])
    mask384 = din("mask384", [128, 384])
    scat384 = din("scat384", [128, 384], dt.int16)
    scat256 = din("scat256", [128, 256], dt.int16)
    cT3r = din("cT3r", [4, NQ * K])
    blockones = din("blockones", [128, 8])      # BO[ch, g] = ch//16 == g
    blockonesT = din("blockonesT", [8, 128])

    out = nc.dram_tensor("out", [128, NQ], dt.float32, kind="ExternalOutput")
    dbg_t = {}
    for name, shape, d in (
        ("dbg_nf", [128, N], dt.float32),
        ("dbg_S", [128, N], dt.float16),
        ("dbg_M8", [128, 128], dt.float16),
        ("dbg_I8", [128, 128], dt.uint16),
        ("dbg_P24", [128, 24], dt.uint16),
        ("dbg_gidx24", [128, 24], dt.uint32),
        ("dbg_gidx16", [128, 16], dt.uint32),
        ("dbg_pos1", [128, NQ], dt.float32),
        ("dbg_stats", [128, 8], dt.float32),
        ("dbg_poshid", [128, NQ], dt.float32),
        ("dbg_attin", [128, NQ], dt.float32),
        ("dbg_att1", [128, NQ], dt.float32),
        ("dbg_e", [128, NQ], dt.float32),
        ("dbg_vg", [128, NQ], dt.float32),
        ("dbg_out1", [128, ntiles * 128], dt.float32),
        ("dbg_post", [128, ntiles * 128], dt.float32),
    ):
        if name in dbg:
            dbg_t[name] = nc.dram_tensor(name, shape, d, kind="ExternalOutput")

    # internal DRAM
    pos1_spill = nc.dram_tensor("pos1_spill", [128, ntiles * NQ], dt.float32)
    att1_spill = nc.dram_tensor("att1_spill", [128, ntiles * NQ], dt.float32)
    out1_spill = nc.dram_tensor("out1_spill", [128, ntiles * 128], dt.float32)
    cc = [(nc.dram_tensor(f"cc{i}_in", [128, 2], dt.float32),
           nc.dram_tensor(f"cc{i}_out", [128, 2], dt.float32)) for i in range(3)]

    COLS = ntiles * NQ * group_size          # spatial cols per batch (N*K when full)
    M_big = 16 * COLS                        # gnorm count (pos/att)
    M_post = 16 * ntiles * 128 * group_size  # gnorm count (post)

    import dataclasses

    def diag_extract(dst, src, nsel):
        """dst[q, c] = src[q, 16*c + q%16]; src [128, 16*nsel], dst [128, nsel]."""
        F = 16 * nsel
        for p8 in range(8):
            sap = dataclasses.replace(src[:], ap=[[F + 16, 16], [16, nsel]],
                                      offset=src[:].offset + p8 * 16 * F)
            dap = dataclasses.replace(dst[:], ap=[[nsel, 16], [1, nsel]],
                                      offset=dst[:].offset + p8 * 16 * nsel)
            nc.sync.dma_start(dap, sap)

    with tile.TileContext(nc) as tc:
        with (
            tc.tile_pool(name="pers", bufs=1) as pers,
            tc.tile_pool(name="work", bufs=1) as work,
            tc.tile_pool(name="big", bufs=4) as bigp,
            tc.tile_pool(name="psA", bufs=2, space="PSUM") as psA,
            tc.tile_pool(name="psB", bufs=2, space="PSUM") as psB,
            tc.tile_pool(name="psT", bufs=1, space="PSUM") as psT,
        ):
            f32, f16, u16, u32, i16 = dt.float32, dt.float16, dt.uint16, dt.uint32, dt.int16

            # ---------- persistent tiles ----------
            NF = pers.tile([128, N], f32, name="NF")
            XR = [pers.tile([128, N], f32, name=f"XR{c}") for c in range(3)]
            WRG = pers.tile([128, ntiles * 128], i16, name="WRG")
            Wt = {n_: pers.tile([128, 128], f32, name="t" + n_) for n_ in wnames}
            L16 = pers.tile([6, 128], f32, name="L16")
            Bt = {n_: pers.tile([128, 1], f32, name="t" + n_) for n_ in bnames}
            IDENT = pers.tile([128, 128], f32, name="IDENT")
            BO = pers.tile([128, 8], f32, name="BO")
            BOT = pers.tile([8, 128], f32, name="BOT")
            CB = pers.tile([128, 128], u32, name="CB")      # chunk base iota
            M0 = pers.tile([128, 384], f32, name="M0")      # refine mask
            PC = pers.tile([NQ, 4], f32, name="PCfull") if False else None
            STAT = pers.tile([128, 2], f32, name="STAT")    # running sums (pos)
            STAT2 = pers.tile([128, 2], f32, name="STAT2")  # (att)
            STAT3 = pers.tile([128, 2], f32, name="STAT3")  # (post)
            EPST = pers.tile([8, 1], f32, name="EPST")

            # ---------- load constants ----------
            for n_ in wnames:
                nc.sync.dma_start(Wt[n_][:], W[n_].ap())
            for n_ in bnames:
                nc.sync.dma_start(Bt[n_][:], B[n_].ap())
            nc.sync.dma_start(L16[:], lhsT6.ap())
            nc.sync.dma_start(IDENT[:], ident.ap())
            nc.sync.dma_start(BO[:], blockones.ap())
            nc.sync.dma_start(BOT[:], blockonesT.ap())
            nc.gpsimd.iota(CB[:], pattern=[[512, 16], [0, 8]], base=0,
                           channel_multiplier=0)
            nc.sync.dma_start(M0[:], mask384.ap())
            SC384 = pers.tile([128, 384], i16, name="SC384")
            SC256 = pers.tile([128, 256], i16, name="SC256")
            nc.sync.dma_start(SC384[:], scat384.ap())
            nc.sync.dma_start(SC256[:], scat256.ap())
            nc.gpsimd.memset(STAT[:], 0.0)
            nc.gpsimd.memset(STAT2[:], 0.0)
            nc.gpsimd.memset(STAT3[:], 0.0)
            nc.gpsimd.memset(EPST[:], EPS)

            # xyz replicated across partitions (doubling broadcast)
            for c in range(3):
                nc.sync.dma_start(XR[c][0:1, :], xyzrow.ap()[c:c + 1, :])
                p = 1
                while p < 128:
                    nc.sync.dma_start(XR[c][p:2 * p, :], XR[c][0:p, :])
                    p *= 2

            # ---------- phase 0: nf ----------
            for c in range(NCH):
                FC = work.tile([128, 512], f32, tag="FCSc", bufs=2, name="FC")
                nc.sync.dma_start(FC[:], feat.ap()[:, c * 512:(c + 1) * 512])
                pb = psA.tile([128, 512], f32, tag="pA", name="pnf")
                nc.tensor.matmul(pb[:], Wt["LWpre"][:], FC[:])
                nc.scalar.activation(NF[:, c * 512:(c + 1) * 512], pb[:],
                                     AF.Identity, bias=Bt["Bpre"][:])
            if "dbg_nf" in dbg_t:
                nc.sync.dma_start(dbg_t["dbg_nf"].ap(), NF[:])

            # ================= PHASE A (per tile) =================
            for t in range(ntiles):
                toff = t * 128
                M8 = work.tile([128, 128], f16, tag="M8", name="M8")
                I8 = work.tile([128, 128], u16, tag="I8", name="I8")
                L13t = work.tile([16, 128], f16, tag="L13t", bufs=2, name="L13t")
                nc.sync.dma_start(L13t[:], lhsT13.ap()[:, toff:toff + 128])
                for c in range(NCH):
                    R13c = work.tile([16, 512], f16, tag="R13c", bufs=2, name="R13c")
                    nc.sync.dma_start(R13c[:], rhs13.ap()[:, c * 512:(c + 1) * 512])
                    pb = psA.tile([128, 512], f32, tag="pA", name="pdist")
                    nc.tensor.matmul(pb[:], L13t[:], R13c[:])
                    Sc = work.tile([128, 512], f16, tag="Sc", bufs=2, name="Sc")
                    nc.scalar.copy(Sc[:], pb[:])
                    nc.vector.max(M8[:, 8 * c:8 * c + 8], Sc[:])
                    nc.vector.max_index(I8[:, 8 * c:8 * c + 8],
                                        M8[:, 8 * c:8 * c + 8], Sc[:])
                    if "dbg_S" in dbg_t and t == 0:
                        nc.sync.dma_start(dbg_t["dbg_S"].ap()[:, c * 512:(c + 1) * 512], Sc[:])
                if "dbg_M8" in dbg_t and t == 0:
                    nc.sync.dma_start(dbg_t["dbg_M8"].ap(), M8[:])
                if "dbg_I8" in dbg_t and t == 0:
                    nc.sync.dma_start(dbg_t["dbg_I8"].ap(), I8[:])

                # Iglob = u32(I8) + 512*chunk
                IG = work.tile([128, 128], u32, tag="IG", name="IG")
                nc.vector.tensor_copy(IG[:], I8[:])
                nc.vector.tensor_tensor(IG[:], IG[:], CB[:], ALU.add)

                # stage 2: top-24 positions of M8
                P24 = work.tile([128, 24], u16, tag="P24", name="P24")
                W8 = work.tile([128, 8], f16, tag="W8", name="W8")
                for r in range(3):
                    nc.vector.max(W8[:], M8[:])
                    nc.vector.max_index(P24[:, 8 * r:8 * r + 8], W8[:], M8[:])
                    if r < 2:
                        nc.vector.match_replace(M8[:], W8[:], M8[:], -F16BIG)
                if "dbg_P24" in dbg_t and t == 0:
                    nc.sync.dma_start(dbg_t["dbg_P24"].ap(), P24[:])

                # gather Iglob at P24 (per-core lists) -> diag extract gidx24
                G384 = work.tile([128, 384], u32, tag="G384", name="G384")
                nc.gpsimd.ap_gather(
                    G384[:], IG[:].rearrange("p (f o) -> p f o", o=1),
                    P24[:].bitcast(i16), channels=128, num_elems=128, d=1,
                    num_idxs=384)
                G384h = work.tile([128, 384], u16, tag="G384h", name="G384h")
                nc.vector.tensor_copy(G384h[:], G384[:])
                GI24w = work.tile([128, 24], u16, tag="GI24w", name="GI24w")
                nc.gpsimd.local_scatter(GI24w[:], G384h[:], SC384[:],
                                        channels=128, num_elems=24, num_idxs=384)
                GI24 = work.tile([128, 24], u32, tag="GI24", name="GI24")
                nc.vector.tensor_copy(GI24[:], GI24w[:])
                if "dbg_gidx24" in dbg_t and t == 0:
                    nc.sync.dma_start(dbg_t["dbg_gidx24"].ap(), GI24[:])

                # refine: gather xyz at candidates, exact d2
                GX = [work.tile([128, 384], f32, tag=f"GX{c}", name=f"GX{c}")
                      for c in range(3)]
                for c in range(3):
                    nc.gpsimd.ap_gather(
                        GX[c][:], XR[c][:].rearrange("p (f o) -> p f o", o=1),
                        GI24w[:].bitcast(i16), channels=128, num_elems=N, d=1,
                        num_idxs=384)
                PCt = work.tile([128, 4], f32, tag="PCt", name="PCt")
                nc.sync.dma_start(PCt[:], pcT.ap()[toff:toff + 128, :])
                SNM = work.tile([128, 384], f32, tag="SNM", name="SNM")
                SQ1 = work.tile([128, 384], f32, tag="SQS", name="SQ1")
                for c in range(3):
                    d_ = GX[c]
                    nc.vector.tensor_tensor(
                        d_[:], d_[:], PCt[:, c:c + 1].broadcast_to([128, 384]),
                        ALU.subtract)
                nc.scalar.activation(SNM[:], GX[0][:], AF.Square)
                nc.scalar.activation(SQ1[:], GX[1][:], AF.Square)
                nc.vector.tensor_tensor(SNM[:], SNM[:], SQ1[:], ALU.add)
                nc.scalar.activation(SQ1[:], GX[2][:], AF.Square)
                nc.vector.tensor_tensor(SNM[:], SNM[:], SQ1[:], ALU.add)
                # snm = M0 - d2  (own positions: -d2; others: -BIG)
                nc.vector.tensor_tensor(SNM[:], M0[:], SNM[:], ALU.subtract)

                P16 = work.tile([128, 16], u16, tag="P16", name="P16")
                W8f = work.tile([128, 8], f32, tag="W8f", name="W8f")
                for r in range(2):
                    nc.vector.max(W8f[:], SNM[:])
                    nc.vector.max_index(P16[:, 8 * r:8 * r + 8], W8f[:], SNM[:])
                    if r < 1:
                        nc.vector.match_replace(SNM[:], W8f[:], SNM[:], -BIG)
                # c16 = P16 >> 4 (position -> candidate rank)
                C16 = work.tile([128, 16], u16, tag="C16", name="C16")
                nc.vector.tensor_scalar(C16[:], P16[:], 4, None,
                                        ALU.logical_shift_right)
                G256 = work.tile([128, 256], u32, tag="G256", name="G256")
                nc.gpsimd.ap_gather(
                    G256[:], GI24[:].rearrange("p (f o) -> p f o", o=1),
                    C16[:].bitcast(i16), channels=128, num_elems=24, d=1,
                    num_idxs=256)
                G256h = work.tile([128, 256], u16, tag="G256h", name="G256h")
                nc.vector.tensor_copy(G256h[:], G256[:])
                GI16w = work.tile([128, 16], u16, tag="GI16w", name="GI16w")
                nc.gpsimd.local_scatter(GI16w[:], G256h[:], SC256[:],
                                        channels=128, num_elems=16, num_idxs=256)
                GI16 = work.tile([128, 16], u32, tag="GI16", name="GI16")
                nc.vector.tensor_copy(GI16[:], GI16w[:])
                if "dbg_gidx16" in dbg_t and t == 0:
                    nc.sync.dma_start(dbg_t["dbg_gidx16"].ap(), GI16[:])

                # wrg slot: transpose(gidx16) replicated x8
                GI16f = work.tile([128, 16], f32, tag="GI16f", name="GI16f")
                nc.vector.tensor_copy(GI16f[:], GI16[:])
                ptr = psT.tile([16, 128], f32, tag="ptr", name="ptr")
                nc.tensor.transpose(ptr[:], GI16f[:], IDENT[:])
                TGf = work.tile([16, 128], f32, tag="TGf", name="TGf")
                nc.scalar.copy(TGf[:], ptr[:])
                wslot = WRG[:, t * 128:(t + 1) * 128]
                nc.vector.tensor_copy(wslot[0:16, :], TGf[:])
                p = 16
                while p < 128:
                    nc.sync.dma_start(wslot[p:2 * p, :], wslot[0:p, :])
                    p *= 2

                # pos1: rhs16 = [xyzg(3); 0; centers(3); 0...]
                PP = bigp.tile([128, NQ], f32, tag="big", name="PP")
                SQS = work.tile([128, 512], f32, tag="SQS", name="SQS")
                A1 = work.tile([128, 1], f32, tag="A1", name="A1")
                A2 = work.tile([128, 1], f32, tag="A2", name="A2")
                for u in range(4):
                    R6 = work.tile([6, 512], f32, tag="R6", bufs=2, name="R6")
                    for c in range(3):
                        XGc = work.tile([16, 512], f32, tag="XGc", bufs=2, name="XGc")
                        nc.gpsimd.ap_gather(
                            XGc[:], XR[c][0:16, :].rearrange("p (f o) -> p f o", o=1),
                            wslot[0:16, 32 * u:32 * u + 32].bitcast(i16),
                            channels=16, num_elems=N, d=1, num_idxs=512)
                        nc.sync.dma_start(R6[c:c + 1, :], XGc[0:1, :])
                    nc.sync.dma_start(
                        R6[3:6, :],
                        cT3r.ap()[0:3, toff * 16 + 512 * u:toff * 16 + 512 * (u + 1)])
                    pb = psB.tile([128, 512], f32, tag="pB", name="ppos1")
                    nc.tensor.matmul(pb[:], L16[:], R6[:])
                    sl = PP[:, u * 512:(u + 1) * 512]
                    nc.scalar.activation(sl, pb[:], AF.Identity,
                                         bias=Bt["Bpos1"][:], accum_out=A1[:])
                    nc.scalar.activation(SQS[:], sl, AF.Square, accum_out=A2[:])
                    nc.vector.tensor_tensor(STAT[:, 0:1], STAT[:, 0:1], A1[:], ALU.add)
                    nc.vector.tensor_tensor(STAT[:, 1:2], STAT[:, 1:2], A2[:], ALU.add)
                nc.sync.dma_start(pos1_spill.ap()[:, t * NQ:(t + 1) * NQ], PP[:])
                if "dbg_pos1" in dbg_t and t == 0:
                    nc.sync.dma_start(dbg_t["dbg_pos1"].ap(), PP[:])

            # ---------- allreduce pos stats + scale/bias ----------
            def allreduce_stats(stat, ccpair, Mcount, Gt, BEt, tag):
                ccin, ccout = ccpair
                nc.sync.dma_start(ccin.ap(), stat[:])
                nc.gpsimd.collective_compute(
                    "AllReduce", ALU.add, replica_groups=groups,
                    ins=[ccin.ap().opt()], outs=[ccout.ap().opt()])
                ST = work.tile([128, 2], f32, tag="ST" + tag, name="ST" + tag)
                nc.sync.dma_start(ST[:], ccout.ap())
                pg = psT.tile([8, 2], f32, tag="pg", name="pg" + tag)
                nc.tensor.matmul(pg[:], BO[:], ST[:])
                GS = work.tile([8, 2], f32, tag="GS" + tag, name="GS" + tag)
                nc.scalar.copy(GS[:], pg[:])
                MM = work.tile([8, 4], f32, tag="MM" + tag, name="MM" + tag)
                nc.vector.tensor_scalar(MM[:, 0:1], GS[:, 0:1], 1.0 / Mcount, None, ALU.mult)
                nc.vector.tensor_scalar(MM[:, 1:2], GS[:, 1:2], 1.0 / Mcount, None, ALU.mult)
                nc.vector.tensor_tensor(MM[:, 2:3], MM[:, 0:1], MM[:, 0:1], ALU.mult)
                nc.vector.tensor_tensor(MM[:, 2:3], MM[:, 1:2], MM[:, 2:3], ALU.subtract)
                # rs = 1/sqrt(var+eps)
                nc.scalar.activation(MM[:, 3:4], MM[:, 2:3], AF.Sqrt, bias=EPST[:])
                nc.vector.reciprocal(MM[:, 3:4], MM[:, 3:4])
                # broadcast to [128,1]
                pr = psT.tile([128, 2], f32, tag="pr", name="pr" + tag)
                nc.tensor.matmul(pr[:, 0:1], BOT[:], MM[:, 3:4])
                nc.tensor.matmul(pr[:, 1:2], BOT[:], MM[:, 0:1])
                SCB = work.tile([128, 2], f32, tag="SCB" + tag, name="SCB" + tag)
                nc.scalar.copy(SCB[:], pr[:])
                SC = work.tile([128, 1], f32, tag="SC" + tag, name="SC" + tag)
                BI = work.tile([128, 1], f32, tag="BI" + tag, name="BI" + tag)
                nc.vector.tensor_tensor(SC[:], SCB[:, 0:1], Gt[:], ALU.mult)
                nc.vector.tensor_tensor(BI[:], SCB[:, 1:2], SC[:], ALU.mult)
                nc.vector.tensor_tensor(BI[:], BEt[:], BI[:], ALU.subtract)
                return SC, BI

            SCp, BIp = allreduce_stats(STAT, cc[0], M_big, Bt["Gpos"], Bt["BEpos"], "p")

            # ================= PHASE B (per tile) =================
            for t in range(ntiles):
                PL = bigp.tile([128, NQ], f32, tag="big", name="PL")
                nc.sync.dma_start(PL[:], pos1_spill.ap()[:, t * NQ:(t + 1) * NQ])
                ZH = bigp.tile([128, NQ], f32, tag="big", name="ZH")
                ZA = bigp.tile([128, NQ], f32, tag="big", name="ZA")
                nc.scalar.activation(ZH[:], PL[:], AF.Identity, bias=BIp[:], scale=SCp[:])
                nc.scalar.activation(ZA[:], PL[:], AF.Abs, bias=BIp[:], scale=SCp[:])
                if "dbg_poshid" in dbg_t and t == 0:
                    nc.sync.dma_start(dbg_t["dbg_poshid"].ap(), ZH[:])
                NFG = bigp.tile([128, NQ], f32, tag="big", name="NFG")
                wslot = WRG[:, t * 128:(t + 1) * 128]
                nc.gpsimd.ap_gather(
                    NFG[:], NF[:].rearrange("p (f o) -> p f o", o=1),
                    wslot.bitcast(i16), channels=128, num_elems=N, d=1, num_idxs=NQ)
                AT = bigp.tile([128, NQ], f32, tag="big", name="AT")
                A1T = bigp.tile([128, NQ], f32, tag="big", name="A1T")
                SQS = work.tile([128, 512], f32, tag="SQS", name="SQSb")
                A1 = work.tile([128, 1], f32, tag="A1", name="A1b")
                A2 = work.tile([128, 1], f32, tag="A2", name="A2b")
                for c in range(4):
                    pb = psB.tile([128, 512], f32, tag="pB", name="pattin")
                    qof = t * 128 + c * 32
                    nc.tensor.matmul(
                        pb[:], Wt["LWq"][:],
                        NF[:, qof:qof + 32].rearrange("p (q o) -> p q o", o=1)
                        .broadcast_to([128, 32, 16]), start=True, stop=False)
                    nc.tensor.matmul(pb[:], Wt["LWkneg"][:],
                                     NFG[:, c * 512:(c + 1) * 512],
                                     start=False, stop=False)
                    nc.tensor.matmul(pb[:], Wt["LWpos2a"][:],
                                     ZH[:, c * 512:(c + 1) * 512],
                                     start=False, stop=False)
                    nc.tensor.matmul(pb[:], Wt["LWpos2b"][:],
                                     ZA[:, c * 512:(c + 1) * 512],
                                     start=False, stop=True)
                    nc.scalar.activation(AT[:, c * 512:(c + 1) * 512], pb[:],
                                         AF.Identity, bias=Bt["Battin"][:])
                    pb2 = psA.tile([128, 512], f32, tag="pA", name="patt1")
                    nc.tensor.matmul(pb2[:], Wt["LWatt1"][:],
                                     AT[:, c * 512:(c + 1) * 512])
                    sl = A1T[:, c * 512:(c + 1) * 512]
                    nc.scalar.activation(sl, pb2[:], AF.Identity,
                                         bias=Bt["Batt1"][:], accum_out=A1[:])
                    nc.scalar.activation(SQS[:], sl, AF.Square, accum_out=A2[:])
                    nc.vector.tensor_tensor(STAT2[:, 0:1], STAT2[:, 0:1], A1[:], ALU.add)
                    nc.vector.tensor_tensor(STAT2[:, 1:2], STAT2[:, 1:2], A2[:], ALU.add)
                nc.sync.dma_start(att1_spill.ap()[:, t * NQ:(t + 1) * NQ], A1T[:])
                if "dbg_attin" in dbg_t and t == 0:
                    nc.sync.dma_start(dbg_t["dbg_attin"].ap(), AT[:])
                if "dbg_att1" in dbg_t and t == 0:
                    nc.sync.dma_start(dbg_t["dbg_att1"].ap(), A1T[:])

            SCa, BIa = allreduce_stats(STAT2, cc[1], M_big, Bt["Gatt"], Bt["BEatt"], "a")

            # ================= PHASE C (per tile) =================
            for t in range(ntiles):
                AL = bigp.tile([128, NQ], f32, tag="big", name="AL")
                nc.sync.dma_start(AL[:], att1_spill.ap()[:, t * NQ:(t + 1) * NQ])
                AFt = bigp.tile([128, NQ], f32, tag="big", name="AFt")
                AFa = bigp.tile([128, NQ], f32, tag="big", name="AFa")
                nc.scalar.activation(AFt[:], AL[:], AF.Identity, bias=BIa[:], scale=SCa[:])
                nc.scalar.activation(AFa[:], AL[:], AF.Abs, bias=BIa[:], scale=SCa[:])
                NFG = bigp.tile([128, NQ], f32, tag="big", name="NFGc")
                wslot = WRG[:, t * 128:(t + 1) * 128]
                nc.gpsimd.ap_gather(
                    NFG[:], NF[:].rearrange("p (f o) -> p f o", o=1),
                    wslot.bitcast(i16), channels=128, num_elems=N, d=1, num_idxs=NQ)
                E = bigp.tile([128, NQ], f32, tag="big", name="E")
                VG = bigp.tile([128, NQ], f32, tag="big", name="VG")
                for c in range(4):
                    pb = psB.tile([128, 512], f32, tag="pB", name="patt2")
                    nc.tensor.matmul(pb[:], Wt["LWatt2a"][:],
                                     AFt[:, c * 512:(c + 1) * 512],
                                     start=True, stop=False)
                    nc.tensor.matmul(pb[:], Wt["LWatt2b"][:],
                                     AFa[:, c * 512:(c + 1) * 512],
                                     start=False, stop=True)
                    nc.scalar.activation(E[:, c * 512:(c + 1) * 512], pb[:],
                                         AF.Exp, bias=Bt["Batt2"][:])
                    pb2 = psA.tile([128, 512], f32, tag="pA", name="pvg")
                    nc.tensor.matmul(pb2[:], Wt["LWv"][:],
                                     NFG[:, c * 512:(c + 1) * 512])
                    nc.scalar.activation(VG[:, c * 512:(c + 1) * 512], pb2[:],
                                         AF.Identity, bias=Bt["Bv"][:])
                if "dbg_e" in dbg_t and t == 0:
                    nc.sync.dma_start(dbg_t["dbg_e"].ap(), E[:])
                if "dbg_vg" in dbg_t and t == 0:
                    nc.sync.dma_start(dbg_t["dbg_vg"].ap(), VG[:])
                SE = work.tile([128, 128], f32, tag="SE", name="SE")
                WS = work.tile([128, 128], f32, tag="WS", name="WS")
                EV = bigp.tile([128, NQ], f32, tag="big", name="EV")
                nc.vector.tensor_reduce(SE[:], E[:].rearrange("p (q j) -> p q j", j=16),
                                        axis=AX.X, op=ALU.add)
                nc.vector.tensor_tensor(EV[:], E[:], VG[:], ALU.mult)
                nc.vector.tensor_reduce(WS[:], EV[:].rearrange("p (q j) -> p q j", j=16),
                                        axis=AX.X, op=ALU.add)
                nc.vector.reciprocal(SE[:], SE[:])
                nc.vector.tensor_tensor(WS[:], WS[:], SE[:], ALU.mult)
                O1t = work.tile([128, 128], f32, tag="O1t", bufs=2, name="O1t")
                nc.vector.tensor_tensor(O1t[:], WS[:],
                                        NF[:, t * 128:(t + 1) * 128], ALU.add)
                nc.sync.dma_start(out1_spill.ap()[:, t * 128:(t + 1) * 128], O1t[:])


            # ---------- post conv + stats ----------
            PST = bigp.tile([128, ntiles * 128], f32, tag="big", name="PST")
            SQS = work.tile([128, 512], f32, tag="SQS", name="SQSp")
            A1 = work.tile([128, 1], f32, tag="A1", name="A1p")
            A2 = work.tile([128, 1], f32, tag="A2", name="A2p")
            npost = ntiles * 128
            for c in range((npost + 511) // 512):
                w = min(512, npost - c * 512)
                OC = work.tile([128, 512], f32, tag="FCSc", bufs=2, name="OC")
                nc.sync.dma_start(OC[:, :w], out1_spill.ap()[:, c * 512:c * 512 + w])
                pb = psB.tile([128, 512], f32, tag="pB", name="ppost")
                nc.tensor.matmul(pb[:, :w], Wt["LWpost"][:], OC[:, :w])
                sl = PST[:, c * 512:c * 512 + w]
                nc.scalar.activation(sl, pb[:, :w], AF.Identity,
                                     bias=Bt["Bpost"][:], accum_out=A1[:])
                nc.scalar.activation(SQS[:, :w], sl, AF.Square, accum_out=A2[:])
                nc.vector.tensor_tensor(STAT3[:, 0:1], STAT3[:, 0:1], A1[:], ALU.add)
                nc.vector.tensor_tensor(STAT3[:, 1:2], STAT3[:, 1:2], A2[:], ALU.add)
            if "dbg_post" in dbg_t:
                nc.sync.dma_start(dbg_t["dbg_post"].ap(), PST[:])

            SCq, BIq = allreduce_stats(STAT3, cc[2], M_post, Bt["Gpost"], Bt["BEpost"], "q")

            # ---------- final: leaky(norm(post)) ----------
            FZ = bigp.tile([128, ntiles * 128], f32, tag="big", name="FZ")
            FA = bigp.tile([128, ntiles * 128], f32, tag="big", name="FA")
            SC055 = work.tile([128, 1], f32, tag="SC055", name="SC055")
            BI055 = work.tile([128, 1], f32, tag="BI055", name="BI055")
            SC045 = work.tile([128, 1], f32, tag="SC045", name="SC045")
            BI045 = work.tile([128, 1], f32, tag="BI045", name="BI045")
            h1, h2 = (1 + NEG) / 2, (1 - NEG) / 2
            nc.vector.tensor_scalar(SC055[:], SCq[:], h1, None, ALU.mult)
            nc.vector.tensor_scalar(BI055[:], BIq[:], h1, None, ALU.mult)
            nc.vector.tensor_scalar(SC045[:], SCq[:], h2, None, ALU.mult)
            nc.vector.tensor_scalar(BI045[:], BIq[:], h2, None, ALU.mult)
            nc.scalar.activation(FZ[:], PST[:], AF.Identity, bias=BI055[:], scale=SC055[:])
            nc.scalar.activation(FA[:], PST[:], AF.Abs, bias=BI045[:], scale=SC045[:])
            nc.vector.tensor_tensor(FZ[:], FZ[:], FA[:], ALU.add)
            nc.sync.dma_start(out.ap()[:, 0:npost], FZ[:])

    nc.compile()
    return nc


NEG_SLOPE = 0.1


def _host_prep(xyz_b, feat_b, W):
    """Per-core inputs from (already rotated) xyz [3,N], feat [128,N]."""
    pts = xyz_b.T.astype(np.float32)
    sq = (pts * pts).sum(-1).astype(np.float32)
    u = (2.0 * pts).astype(np.float32)
    uhi = u.astype(np.float16)
    ulo = (u - uhi.astype(np.float32)).astype(np.float16)
    phi = pts.astype(np.float16)
    plo = (pts - phi.astype(np.float32)).astype(np.float16)
    shi = sq.astype(np.float16)
    slo = (sq - shi.astype(np.float32)).astype(np.float16)

    rhs13 = np.zeros((16, N), np.float16)
    rhs13[0:3] = phi.T
    rhs13[3:6] = plo.T
    rhs13[6:9] = phi.T
    rhs13[9] = -np.ones(N, np.float16)
    rhs13[10] = -np.ones(N, np.float16)
    rhs13[11] = -shi
    rhs13[12] = -slo

    qsl = slice(0, NQ)
    lhsT13 = np.zeros((16, NQ), np.float16)
    lhsT13[0:3] = uhi[qsl].T
    lhsT13[3:6] = uhi[qsl].T
    lhsT13[6:9] = ulo[qsl].T
    lhsT13[9] = shi[qsl]
    lhsT13[10] = slo[qsl]
    lhsT13[11] = np.ones(NQ, np.float16)
    lhsT13[12] = np.ones(NQ, np.float16)

    cT3 = np.zeros((4, NQ), np.float32)
    cT3[0:3] = xyz_b[:, qsl]
    pc = np.zeros((NQ, 4), np.float32)
    pc[:, 0:3] = pts[qsl]
    xyzrow = np.zeros((4, N), np.float32)
    xyzrow[0:3] = xyz_b
    cT3r = np.zeros((4, NQ * K), np.float32)
    cT3r[0:3] = np.repeat(cT3[0:3], K, axis=1)
    mask384 = np.full((128, 384), -BIG, np.float32)
    scat384 = np.full((128, 384), -1, np.int16)
    scat256 = np.full((128, 256), -1, np.int16)
    for q in range(128):
        mask384[q, q % 16::16] = 0.0
        scat384[q, q % 16::16] = np.arange(24, dtype=np.int16)
        scat256[q, q % 16::16] = np.arange(16, dtype=np.int16)
    lt = lambda m: np.ascontiguousarray(m.T)
    h1, h2 = (1 + NEG_SLOPE) / 2, (1 - NEG_SLOPE) / 2
    bo = np.zeros((128, 8), np.float32)
    for ch in range(128):
        bo[ch, ch // 16] = 1.0
    d = {
        "feat": feat_b.astype(np.float32),
        "rhs13": rhs13, "lhsT13": lhsT13, "cT3": cT3, "pcT": pc,
        "xyzrow": xyzrow, "cT3r": cT3r, "mask384": mask384,
        "scat384": scat384, "scat256": scat256,
        "LWpre": lt(W["W_pre"]), "LWq": lt(W["W_q"]),
        "LWkneg": lt(-W["W_k"]), "LWv": lt(W["W_v"]),
        "LWpos2a": lt(W["W_pos2"]) * h1, "LWpos2b": lt(W["W_pos2"]) * h2,
        "LWatt1": lt(W["W_att1"]),
        "LWatt2a": lt(W["W_att2"]) * h1, "LWatt2b": lt(W["W_att2"]) * h2,
        "LWpost": lt(W["W_post"]),
        "lhsT6": np.concatenate([W["W_pos1"].T, -W["W_pos1"].T]).astype(np.float32),
        "Bpre": W["b_pre"].reshape(128, 1),
        "Bv": W["b_v"].reshape(128, 1),
        "Battin": (W["b_q"] - W["b_k"] + W["b_pos2"]).reshape(128, 1),
        "Batt1": W["b_att1"].reshape(128, 1),
        "Batt2": W["b_att2"].reshape(128, 1),
        "Bpost": W["b_post"].reshape(128, 1),
        "Bpos1": W["b_pos1"].reshape(128, 1),
        "Gpos": W["g_pos1"].reshape(128, 1),
        "BEpos": W["be_pos1"].reshape(128, 1),
        "Gatt": W["g_att1"].reshape(128, 1),
        "BEatt": W["be_att1"].reshape(128, 1),
        "Gpost": W["g_post"].reshape(128, 1),
        "BEpost": W["be_post"].reshape(128, 1),
        "ident": np.eye(128, dtype=np.float32),
        "blockones": bo,
        "blockonesT": np.ascontiguousarray(bo.T),
    }
    return {k: np.ascontiguousarray(np.asarray(v, dtype=v.dtype if hasattr(v, "dtype") else np.float32)) for k, v in d.items()}


_CACHE = {}


def kernel(**inputs) -> np.ndarray:
    xyz = np.asarray(inputs["xyz"], np.float32)    # [2, 3, 8192]
    feat = np.asarray(inputs["feat"], np.float32)  # [2, 128, 8192]
    W = {k: np.asarray(v, np.float32) for k, v in inputs.items()
         if k not in ("xyz", "feat")}

    if "nc" not in _CACHE:
        _CACHE["nc"] = build(n_cores=8, ntiles=16)
    nc = _CACHE["nc"]

    in_maps = []
    for c in range(8):
        b, qs = c // 4, (c % 4) * NQ
        rot = np.roll(np.arange(N), -qs)
        in_maps.append(_host_prep(xyz[b][:, rot], feat[b][:, rot], W))

    res = run_bass_kernel_spmd(nc, in_maps, list(range(8)))
    outp = np.zeros((2, 128, N), np.float32)
    for c in range(8):
        b, qs = c // 4, (c % 4) * NQ
        outp[b][:, qs:qs + NQ] = res.results[c]["out"]
    return outp

